# revision 15
# baseline (speedup 1.0000x reference)
"""Trainium2 Bass kernel for BlockGivensRotation (w @ R, block-diagonal).

The reference applies, per 128-column block of w, 8 sequential sweeps of 127
adjacent-plane Givens rotations.  The composition of all 1016 rotations of a
block is a fixed 128x128 orthogonal matrix R_nb that depends only on `angles`,
so the whole op is `out[:, nb*128:(nb+1)*128] = w[:, nb*128:(nb+1)*128] @ R_nb`
- a block-diagonal matmul, ideal for the tensor engine.

Host side: compose R (tiny: 64x128x128, built in f64 from the 65K angles).
Device side: shard the 64 column-blocks across the 8 cores (8 blocks each).

The kernel is DMA-bound (the 8 cores together sit at the chip HBM roofline),
so bytes moved is the metric that matters.  Two tricks cut it 4x vs f32:

1. Delta form: R = I + (R-I), so out = w + w@(R-I).  The device computes only
   delta = w@(R-I); the host adds w back in f32.  Every quantization error on
   the device path is then scaled by ||R-I||_F/sqrt(128) ~= 0.39 instead of 1.
2. fp8 I/O: w streams in as float8_e3m4 (1 byte), delta streams out as
   float8_e3m4, with the stationary R-I in fp16 and f32 PSUM accumulation.
   Measured end-to-end rel err ~8e-3 against the f32 reference (gate: 2e-2).

Each core streams w.T tiles from DRAM (SP ring), matmuls with the per-block
stationary R-I, copies PSUM->SBUF with the copy work round-robined over the
vector/scalar/pool engines (one engine alone would be the bottleneck), and
writes delta.T tiles back on the ACT ring.  w is fed transposed so the
contraction dim (block columns) lies on SBUF partitions with fully contiguous
DMA; the host transposes shards in/out.
"""

import numpy as np
import ml_dtypes

import concourse.bacc as bacc
import concourse.mybir as mybir
import concourse.tile as tile
from concourse.bass_utils import run_bass_kernel_spmd

O = 8192          # w rows
IN_F = 8192       # w cols
B = 128           # Givens block size
NB = IN_F // B    # 64 blocks
NCORES = 8
BPC = NB // NCORES  # 8 column-blocks per core
F32 = mybir.dt.float32

# Device dtypes (mybir, numpy) and mode.  delta=True runs the R-I form with
# the host adding w back; delta=False computes w@R directly on device.
W_DT = mybir.dt.float8e3
R_DT = mybir.dt.float16
OUT_DT = mybir.dt.float8e3
DELTA = True

_NP = {
    mybir.dt.float8e3: ml_dtypes.float8_e3m4,
    mybir.dt.float8e4: ml_dtypes.float8_e4m3,
    mybir.dt.float16: np.float16,
    mybir.dt.bfloat16: ml_dtypes.bfloat16,
    mybir.dt.float32: np.float32,
}


def _build_rotation_matrices(angles: np.ndarray) -> np.ndarray:
    """Compose the sweeps of adjacent Givens rotations into one 128x128
    matrix per block by applying the reference recurrence to the identity
    (in float64, rounded once at the end)."""
    nb, s, bm1 = angles.shape
    b = bm1 + 1
    ang = np.asarray(angles, dtype=np.float64)
    c = np.cos(ang)
    sn = np.sin(ang)
    R = np.broadcast_to(np.eye(b), (nb, b, b)).copy()  # [NB, basis row, col]
    for sweep in range(s):
        cs, ss = c[:, sweep, :], sn[:, sweep, :]
        carry = R[:, :, 0].copy()
        for i in range(bm1):
            col_j = R[:, :, i + 1]
            ci = cs[:, i][:, None]
            si = ss[:, i][:, None]
            R[:, :, i] = ci * carry - si * col_j
            carry = si * carry + ci * col_j
        R[:, :, b - 1] = carry
    return R


def _build_bass(
    rows=O,
    bpc=BPC,
    ncores=NCORES,
    tile_rows=8192,
    wt_bufs=6,
    out_bufs=4,
    r_first=2,
    first_segs=(512, 3584, 4096),
    last_segs=(4096, 3584, 512),
    copy_engines=("scalar", "vector"),
    w_dt=W_DT,
    r_dt=R_DT,
    out_dt=OUT_DT,
    group_banks=2,
):
    """Per-core program over this core's `bpc` column-blocks of w:

        out_t[blk*B + c', r] = sum_c Rmi[blk][c, c'] * wt[blk*B + c, r]

    rows: w rows (full, 8192); tile_rows: rows per DMA tile;
    wt_bufs/out_bufs: pipeline depth; r_first: blocks of R in the first
    (small) R chunk so the first matmul isn't gated on the whole R slice;
    first_segs: tapered tiles for the first block so the PE starts on a
    tiny tile; last_segs: tapered tiles for the last block so the final
    store (which the kernel-end barrier waits on) is tiny;
    copy_engines: round-robin assignment for the PSUM->SBUF copies;
    group_banks: PSUM banks per ping-pong group tile.  The 8 PSUM banks
    are split into `8//group_banks` group tiles; each group runs
    `group_banks` back-to-back matmuls (hiding the ~173ns PE pipe-fill)
    gated on the WHOLE tile being drained, while the copy engines drain
    the other tile.  Whole-tile gating keeps the PE in burst mode - with
    per-bank gating the pipeline degenerates into a stable trickle where
    every matmul is released by a single copy and re-pays the pipe-fill.
    """
    nc = bacc.Bacc(
        "TRN2", target_bir_lowering=False, debug=False, num_devices=ncores
    )
    wt = nc.dram_tensor("wt", [bpc * B, rows], w_dt, kind="ExternalInput")
    r = nc.dram_tensor("r", [B, bpc * B], r_dt, kind="ExternalInput")
    out_t = nc.dram_tensor("out_t", [bpc * B, rows], out_dt, kind="ExternalOutput")

    hs = 512                    # moving free-dim per matmul (one PSUM bank f32)
    gs = hs * group_banks       # free-dim per PSUM group tile

    with tile.TileContext(nc) as tc:
        with (
            tc.tile_pool(name="rp", bufs=1) as rp,
            tc.tile_pool(name="wtp", bufs=wt_bufs) as wtp,
            tc.tile_pool(name="outp", bufs=out_bufs) as outp,
            tc.tile_pool(name="psp", bufs=8 // group_banks, space="PSUM") as psp,
        ):
            # The ACT-ring sequencer exits the init barrier ~1.2us before
            # SP's, so the latency-critical first bytes (the first R chunk
            # and a tiny first w tile) go on ACT; the bulk w stream rides
            # SP and the stores ride ACT.
            rf = min(r_first, bpc)
            r_a = rp.tile([B, rf * B], r_dt, tag="ra")
            nc.scalar.dma_start(r_a[:], r[:, : rf * B])
            r_b = None
            if rf < bpc:
                r_b = rp.tile([B, (bpc - rf) * B], r_dt, tag="rb")
                nc.scalar.dma_start(r_b[:], r[:, rf * B :])
            ncopy = 0
            for blk in range(bpc):
                if blk < rf:
                    r_ap = r_a[:, blk * B : (blk + 1) * B]
                else:
                    r_ap = r_b[:, (blk - rf) * B : (blk - rf + 1) * B]
                if blk == 0 and first_segs:
                    sizes = list(first_segs)
                elif blk == bpc - 1 and last_segs:
                    sizes = list(last_segs)
                else:
                    sizes = [
                        min(tile_rows, rows - o) for o in range(0, rows, tile_rows)
                    ]
                assert sum(sizes) == rows
                segs, o = [], 0
                for sz in sizes:
                    segs.append((o, sz))
                    o += sz
                for si, (o, seg) in enumerate(segs):
                    wt_tile = wtp.tile([B, seg], w_dt, tag="wt")
                    nc.sync.dma_start(
                        wt_tile[:], wt[blk * B : (blk + 1) * B, o : o + seg]
                    )
                    out_tile = outp.tile([B, seg], out_dt, tag="out")
                    # Ping-pong over PSUM group tiles: burst the group's
                    # matmuls back-to-back, then drain with copies
                    # alternating between scalar and vector.
                    for g0 in range(0, seg, gs):
                        gw = min(gs, seg - g0)
                        ps = psp.tile([B, gs], F32)
                        chunks = []
                        for h0 in range(0, gw, hs):
                            hw_ = min(hs, gw - h0)
                            chunks.append((h0, hw_))
                            nc.tensor.matmul(
                                ps[:, h0 : h0 + hw_],
                                r_ap,
                                wt_tile[:, g0 + h0 : g0 + h0 + hw_],
                                start=True,
                                stop=True,
                            )
                        for h0, hw_ in chunks:
                            eng = copy_engines[ncopy % len(copy_engines)]
                            ncopy += 1
                            dst = out_tile[:, g0 + h0 : g0 + h0 + hw_]
                            src = ps[:, h0 : h0 + hw_]
                            if eng == "vector":
                                nc.vector.tensor_copy(dst, src)
                            else:
                                nc.scalar.copy(dst, src)
                    # out-stores ride the ACT HWDGE ring
                    nc.scalar.dma_start(
                        out_t[blk * B : (blk + 1) * B, o : o + seg], out_tile[:]
                    )
    nc.compile()
    return nc


def kernel_impl(w, angles, trace=False, bass_kwargs=None, **spmd_kwargs):
    w = np.asarray(w)
    bass_kwargs = dict(bass_kwargs or {})
    w_dt = bass_kwargs.setdefault("w_dt", W_DT)
    r_dt = bass_kwargs.setdefault("r_dt", R_DT)
    out_dt = bass_kwargs.setdefault("out_dt", OUT_DT)
    delta = bass_kwargs.pop("delta", DELTA)

    Rm = _build_rotation_matrices(np.asarray(angles))  # [NB, B, B] f64
    if delta:
        Rm = Rm - np.eye(B)
    # r_host[c, blk*B + c'] = Rm[blk][c, c']  (contiguous per SBUF partition c)
    r_host = (
        np.ascontiguousarray(Rm.transpose(1, 0, 2)).reshape(B, NB * B)
        .astype(_NP[r_dt])
    )
    nc = _build_bass(**bass_kwargs)
    csz = BPC * B  # 1024 w-columns per core
    in_maps = [
        {
            "wt": np.ascontiguousarray(w[:, i * csz : (i + 1) * csz].T).astype(
                _NP[w_dt]
            ),
            "r": r_host[:, i * csz : (i + 1) * csz],
        }
        for i in range(NCORES)
    ]
    res = run_bass_kernel_spmd(
        nc, in_maps, core_ids=list(range(NCORES)), trace=trace, **spmd_kwargs
    )
    out = np.empty((O, IN_F), dtype=np.float32)
    for i in range(NCORES):
        sl = slice(i * csz, (i + 1) * csz)
        dev = np.asarray(res.results[i]["out_t"]).astype(np.float32).T
        out[:, sl] = (w[:, sl] + dev) if delta else dev
    return out, res


def kernel(w, angles):
    out, _ = kernel_impl(w, angles, trace=False)
    return out


# revision 17
# speedup vs baseline: 1.0015x; 1.0015x over previous
"""Trainium2 Bass kernel for BlockGivensRotation (w @ R, block-diagonal).

The reference applies, per 128-column block of w, 8 sequential sweeps of 127
adjacent-plane Givens rotations.  The composition of all 1016 rotations of a
block is a fixed 128x128 orthogonal matrix R_nb that depends only on `angles`,
so the whole op is `out[:, nb*128:(nb+1)*128] = w[:, nb*128:(nb+1)*128] @ R_nb`
- a block-diagonal matmul, ideal for the tensor engine.

Host side: compose R (tiny: 64x128x128, built in f64 from the 65K angles).
Device side: shard the 64 column-blocks across the 8 cores (8 blocks each).

The kernel is DMA-bound (the 8 cores together sit at the chip HBM roofline),
so bytes moved is the metric that matters.  Two tricks cut it 4x vs f32:

1. Delta form: R = I + (R-I), so out = w + w@(R-I).  The device computes only
   delta = w@(R-I); the host adds w back in f32.  Every quantization error on
   the device path is then scaled by ||R-I||_F/sqrt(128) ~= 0.39 instead of 1.
2. fp8 I/O: w streams in as float8_e3m4 (1 byte), delta streams out as
   float8_e3m4, with the stationary R-I in fp16 and f32 PSUM accumulation.
   Measured end-to-end rel err ~8e-3 against the f32 reference (gate: 2e-2).

Each core streams w.T tiles from DRAM (SP ring), matmuls with the per-block
stationary R-I, copies PSUM->SBUF with the copy work round-robined over the
vector/scalar/pool engines (one engine alone would be the bottleneck), and
writes delta.T tiles back on the ACT ring.  w is fed transposed so the
contraction dim (block columns) lies on SBUF partitions with fully contiguous
DMA; the host transposes shards in/out.
"""

import numpy as np
import ml_dtypes

import concourse.bacc as bacc
import concourse.mybir as mybir
import concourse.tile as tile
from concourse.bass_utils import run_bass_kernel_spmd

O = 8192          # w rows
IN_F = 8192       # w cols
B = 128           # Givens block size
NB = IN_F // B    # 64 blocks
NCORES = 8
BPC = NB // NCORES  # 8 column-blocks per core
F32 = mybir.dt.float32

# Device dtypes (mybir, numpy) and mode.  delta=True runs the R-I form with
# the host adding w back; delta=False computes w@R directly on device.
W_DT = mybir.dt.float8e3
R_DT = mybir.dt.float16
OUT_DT = mybir.dt.float8e3
DELTA = True

_NP = {
    mybir.dt.float8e3: ml_dtypes.float8_e3m4,
    mybir.dt.float8e4: ml_dtypes.float8_e4m3,
    mybir.dt.float16: np.float16,
    mybir.dt.bfloat16: ml_dtypes.bfloat16,
    mybir.dt.float32: np.float32,
}


def _build_rotation_matrices(angles: np.ndarray) -> np.ndarray:
    """Compose the sweeps of adjacent Givens rotations into one 128x128
    matrix per block by applying the reference recurrence to the identity
    (in float64, rounded once at the end)."""
    nb, s, bm1 = angles.shape
    b = bm1 + 1
    ang = np.asarray(angles, dtype=np.float64)
    c = np.cos(ang)
    sn = np.sin(ang)
    R = np.broadcast_to(np.eye(b), (nb, b, b)).copy()  # [NB, basis row, col]
    for sweep in range(s):
        cs, ss = c[:, sweep, :], sn[:, sweep, :]
        carry = R[:, :, 0].copy()
        for i in range(bm1):
            col_j = R[:, :, i + 1]
            ci = cs[:, i][:, None]
            si = ss[:, i][:, None]
            R[:, :, i] = ci * carry - si * col_j
            carry = si * carry + ci * col_j
        R[:, :, b - 1] = carry
    return R


def _build_bass(
    rows=O,
    bpc=BPC,
    ncores=NCORES,
    tile_rows=8192,
    wt_bufs=6,
    out_bufs=4,
    r_first=2,
    first_segs=(512, 3584, 4096),
    last_segs=(4096, 3584, 512),
    copy_engines=("scalar", "vector"),
    w_dt=W_DT,
    r_dt=R_DT,
    out_dt=OUT_DT,
    group_banks=2,
    store_engine="scalar",
):
    """Per-core program over this core's `bpc` column-blocks of w:

        out_t[blk*B + c', r] = sum_c Rmi[blk][c, c'] * wt[blk*B + c, r]

    rows: w rows (full, 8192); tile_rows: rows per DMA tile;
    wt_bufs/out_bufs: pipeline depth; r_first: blocks of R in the first
    (small) R chunk so the first matmul isn't gated on the whole R slice;
    first_segs: tapered tiles for the first block so the PE starts on a
    tiny tile; last_segs: tapered tiles for the last block so the final
    store (which the kernel-end barrier waits on) is tiny;
    copy_engines: round-robin assignment for the PSUM->SBUF copies;
    group_banks: PSUM banks per ping-pong group tile.  The 8 PSUM banks
    are split into `8//group_banks` group tiles; each group runs
    `group_banks` back-to-back matmuls (hiding the ~173ns PE pipe-fill)
    gated on the WHOLE tile being drained, while the copy engines drain
    the other tile.  Whole-tile gating keeps the PE in burst mode - with
    per-bank gating the pipeline degenerates into a stable trickle where
    every matmul is released by a single copy and re-pays the pipe-fill.
    """
    nc = bacc.Bacc(
        "TRN2", target_bir_lowering=False, debug=False, num_devices=ncores
    )
    wt = nc.dram_tensor("wt", [bpc * B, rows], w_dt, kind="ExternalInput")
    r = nc.dram_tensor("r", [B, bpc * B], r_dt, kind="ExternalInput")
    out_t = nc.dram_tensor("out_t", [bpc * B, rows], out_dt, kind="ExternalOutput")

    hs = 512                    # moving free-dim per matmul (one PSUM bank f32)
    gs = hs * group_banks       # free-dim per PSUM group tile

    with tile.TileContext(nc) as tc:
        with (
            tc.tile_pool(name="rp", bufs=1) as rp,
            tc.tile_pool(name="wtp", bufs=wt_bufs) as wtp,
            tc.tile_pool(name="outp", bufs=out_bufs) as outp,
            tc.tile_pool(name="psp", bufs=8 // group_banks, space="PSUM") as psp,
        ):
            # The ACT-ring sequencer exits the init barrier ~1.2us before
            # SP's, so the latency-critical first bytes (the first R chunk
            # and a tiny first w tile) go on ACT; the bulk w stream rides
            # SP and the stores ride ACT.
            rf = min(r_first, bpc)
            r_a = rp.tile([B, rf * B], r_dt, tag="ra")
            nc.scalar.dma_start(r_a[:], r[:, : rf * B])
            r_b = None
            if rf < bpc:
                r_b = rp.tile([B, (bpc - rf) * B], r_dt, tag="rb")
                nc.scalar.dma_start(r_b[:], r[:, rf * B :])
            ncopy = 0
            for blk in range(bpc):
                if blk < rf:
                    r_ap = r_a[:, blk * B : (blk + 1) * B]
                else:
                    r_ap = r_b[:, (blk - rf) * B : (blk - rf + 1) * B]
                if blk == 0 and first_segs:
                    sizes = list(first_segs)
                elif blk == bpc - 1 and last_segs:
                    sizes = list(last_segs)
                else:
                    sizes = [
                        min(tile_rows, rows - o) for o in range(0, rows, tile_rows)
                    ]
                assert sum(sizes) == rows
                segs, o = [], 0
                for sz in sizes:
                    segs.append((o, sz))
                    o += sz
                for si, (o, seg) in enumerate(segs):
                    wt_tile = wtp.tile([B, seg], w_dt, tag="wt")
                    nc.sync.dma_start(
                        wt_tile[:], wt[blk * B : (blk + 1) * B, o : o + seg]
                    )
                    out_tile = outp.tile([B, seg], out_dt, tag="out")
                    # Ping-pong over PSUM group tiles: burst the group's
                    # matmuls back-to-back, then drain with copies
                    # alternating between scalar and vector.
                    for g0 in range(0, seg, gs):
                        gw = min(gs, seg - g0)
                        ps = psp.tile([B, gs], F32)
                        chunks = []
                        for h0 in range(0, gw, hs):
                            hw_ = min(hs, gw - h0)
                            chunks.append((h0, hw_))
                            nc.tensor.matmul(
                                ps[:, h0 : h0 + hw_],
                                r_ap,
                                wt_tile[:, g0 + h0 : g0 + h0 + hw_],
                                start=True,
                                stop=True,
                            )
                        for h0, hw_ in chunks:
                            eng = copy_engines[ncopy % len(copy_engines)]
                            ncopy += 1
                            dst = out_tile[:, g0 + h0 : g0 + h0 + hw_]
                            src = ps[:, h0 : h0 + hw_]
                            if eng == "vector":
                                nc.vector.tensor_copy(dst, src)
                            else:
                                nc.scalar.copy(dst, src)
                    # out-stores ride a HWDGE ring (ACT by default)
                    st_eng = nc.sync if store_engine == "sync" else nc.scalar
                    st_eng.dma_start(
                        out_t[blk * B : (blk + 1) * B, o : o + seg], out_tile[:]
                    )
    nc.compile()
    return nc


def kernel_impl(w, angles, trace=False, bass_kwargs=None, **spmd_kwargs):
    w = np.asarray(w)
    bass_kwargs = dict(bass_kwargs or {})
    w_dt = bass_kwargs.setdefault("w_dt", W_DT)
    r_dt = bass_kwargs.setdefault("r_dt", R_DT)
    out_dt = bass_kwargs.setdefault("out_dt", OUT_DT)
    delta = bass_kwargs.pop("delta", DELTA)

    Rm = _build_rotation_matrices(np.asarray(angles))  # [NB, B, B] f64
    if delta:
        Rm = Rm - np.eye(B)
    # r_host[c, blk*B + c'] = Rm[blk][c, c']  (contiguous per SBUF partition c)
    r_host = (
        np.ascontiguousarray(Rm.transpose(1, 0, 2)).reshape(B, NB * B)
        .astype(_NP[r_dt])
    )
    nc = _build_bass(**bass_kwargs)
    csz = BPC * B  # 1024 w-columns per core
    in_maps = [
        {
            "wt": np.ascontiguousarray(w[:, i * csz : (i + 1) * csz].T).astype(
                _NP[w_dt]
            ),
            "r": r_host[:, i * csz : (i + 1) * csz],
        }
        for i in range(NCORES)
    ]
    res = run_bass_kernel_spmd(
        nc, in_maps, core_ids=list(range(NCORES)), trace=trace, **spmd_kwargs
    )
    out = np.empty((O, IN_F), dtype=np.float32)
    for i in range(NCORES):
        sl = slice(i * csz, (i + 1) * csz)
        dev = np.asarray(res.results[i]["out_t"]).astype(np.float32).T
        out[:, sl] = (w[:, sl] + dev) if delta else dev
    return out, res


def kernel(w, angles):
    out, _ = kernel_impl(w, angles, trace=False)
    return out


# revision 23
# speedup vs baseline: 1.0123x; 1.0108x over previous
"""Trainium2 Bass kernel for BlockGivensRotation (w @ R, block-diagonal).

The reference applies, per 128-column block of w, 8 sequential sweeps of 127
adjacent-plane Givens rotations.  The composition of all 1016 rotations of a
block is a fixed 128x128 orthogonal matrix R_nb that depends only on `angles`,
so the whole op is `out[:, nb*128:(nb+1)*128] = w[:, nb*128:(nb+1)*128] @ R_nb`
- a block-diagonal matmul, ideal for the tensor engine.

Host side: compose R (tiny: 64x128x128, built in f64 from the 65K angles).
Device side: shard the 64 column-blocks across the 8 cores (8 blocks each).

The kernel is DMA-bound (the 8 cores together sit at the chip HBM roofline),
so bytes moved is the metric that matters.  Two tricks cut it 4x vs f32:

1. Delta form: R = I + (R-I), so out = w + w@(R-I).  The device computes only
   delta = w@(R-I); the host adds w back in f32.  Every quantization error on
   the device path is then scaled by ||R-I||_F/sqrt(128) ~= 0.39 instead of 1.
2. fp8 I/O: w streams in as float8_e3m4 (1 byte), delta streams out as
   float8_e3m4, with the stationary R-I in fp16 and f32 PSUM accumulation.
   Measured end-to-end rel err ~8e-3 against the f32 reference (gate: 2e-2).

Each core streams w.T tiles from DRAM (SP ring), matmuls with the per-block
stationary R-I, copies PSUM->SBUF with the copy work alternated between the
vector and scalar engines (the only two that can read PSUM; one alone would
be the bottleneck), and writes delta.T tiles back on the ACT ring.  w is fed
transposed so the contraction dim (block columns) lies on SBUF partitions
with fully contiguous DMA; the host transposes shards in/out.
"""

import numpy as np
import ml_dtypes

import concourse.bacc as bacc
import concourse.mybir as mybir
import concourse.tile as tile
from concourse.bass_utils import run_bass_kernel_spmd

O = 8192          # w rows
IN_F = 8192       # w cols
B = 128           # Givens block size
NB = IN_F // B    # 64 blocks
NCORES = 8
BPC = NB // NCORES  # 8 column-blocks per core
F32 = mybir.dt.float32

# Device dtypes (mybir, numpy) and mode.  delta=True runs the R-I form with
# the host adding w back; delta=False computes w@R directly on device.
W_DT = mybir.dt.float8e3
R_DT = mybir.dt.float16
OUT_DT = mybir.dt.float8e3
DELTA = True

_NP = {
    mybir.dt.float8e3: ml_dtypes.float8_e3m4,
    mybir.dt.float8e4: ml_dtypes.float8_e4m3,
    mybir.dt.float16: np.float16,
    mybir.dt.bfloat16: ml_dtypes.bfloat16,
    mybir.dt.float32: np.float32,
}


def _build_rotation_matrices(angles: np.ndarray) -> np.ndarray:
    """Compose the sweeps of adjacent Givens rotations into one 128x128
    matrix per block by applying the reference recurrence to the identity
    (in float64, rounded once at the end)."""
    nb, s, bm1 = angles.shape
    b = bm1 + 1
    ang = np.asarray(angles, dtype=np.float64)
    c = np.cos(ang)
    sn = np.sin(ang)
    R = np.broadcast_to(np.eye(b), (nb, b, b)).copy()  # [NB, basis row, col]
    for sweep in range(s):
        cs, ss = c[:, sweep, :], sn[:, sweep, :]
        carry = R[:, :, 0].copy()
        for i in range(bm1):
            col_j = R[:, :, i + 1]
            ci = cs[:, i][:, None]
            si = ss[:, i][:, None]
            R[:, :, i] = ci * carry - si * col_j
            carry = si * carry + ci * col_j
        R[:, :, b - 1] = carry
    return R


def _build_bass(
    rows=O,
    bpc=BPC,
    ncores=NCORES,
    tile_rows=8192,
    wt_bufs=6,
    out_bufs=4,
    r_first=2,
    first_segs=(512, 3584, 4096),
    last_segs=(4096, 3584, 512),
    copy_engines=("vector", "scalar"),
    w_dt=W_DT,
    r_dt=R_DT,
    out_dt=OUT_DT,
    group_banks=2,
    copy_banks=1,
    out_rows=None,
    store_engine="scalar",
):
    """Per-core program over this core's `bpc` column-blocks of w:

        out_t[blk*B + c', r] = sum_c Rmi[blk][c, c'] * wt[blk*B + c, r]

    rows: w rows (full, 8192); tile_rows: rows per DMA tile;
    wt_bufs/out_bufs: pipeline depth; r_first: blocks of R in the first
    (small) R chunk so the first matmul isn't gated on the whole R slice;
    first_segs: tapered tiles for the first block so the PE starts on a
    tiny tile; last_segs: tapered tiles for the last block so the final
    store (which the kernel-end barrier waits on) is tiny;
    copy_engines: round-robin assignment for the PSUM->SBUF copies;
    group_banks: PSUM banks per ping-pong group tile.  The 8 PSUM banks
    are split into `8//group_banks` group tiles; each group runs
    `group_banks` back-to-back matmuls (hiding the ~173ns PE pipe-fill)
    gated on the WHOLE tile being drained, while the copy engines drain
    the other tile.  Whole-tile gating keeps the PE in burst mode - with
    per-bank gating the pipeline degenerates into a stable trickle where
    every matmul is released by a single copy and re-pays the pipe-fill.
    """
    nc = bacc.Bacc(
        "TRN2", target_bir_lowering=False, debug=False, num_devices=ncores
    )
    wt = nc.dram_tensor("wt", [bpc * B, rows], w_dt, kind="ExternalInput")
    r = nc.dram_tensor("r", [B, bpc * B], r_dt, kind="ExternalInput")
    out_t = nc.dram_tensor("out_t", [bpc * B, rows], out_dt, kind="ExternalOutput")

    hs = 512                    # moving free-dim per matmul (one PSUM bank f32)
    gs = hs * group_banks       # free-dim per PSUM group tile
    cs = hs * copy_banks        # free-dim per PSUM->SBUF copy

    with tile.TileContext(nc) as tc:
        with (
            tc.tile_pool(name="rp", bufs=1) as rp,
            tc.tile_pool(name="wtp", bufs=wt_bufs) as wtp,
            tc.tile_pool(name="outp", bufs=out_bufs) as outp,
            tc.tile_pool(name="psp", bufs=8 // group_banks, space="PSUM") as psp,
        ):
            # R rides the ACT ring (its sequencer exits the init barrier
            # ~1.2us before SP's); the w stream rides SP with the tiny
            # first tile first in issue order - descriptors from both
            # rings share the 16 hardware DMA queues, so anything issued
            # ahead of it would delay the first matmul.
            rf = min(r_first, bpc)
            r_a = rp.tile([B, rf * B], r_dt, tag="ra")
            nc.scalar.dma_start(r_a[:], r[:, : rf * B])
            r_b = None
            if rf < bpc:
                r_b = rp.tile([B, (bpc - rf) * B], r_dt, tag="rb")
                nc.scalar.dma_start(r_b[:], r[:, rf * B :])
            ncopy = 0
            for blk in range(bpc):
                if blk < rf:
                    r_ap = r_a[:, blk * B : (blk + 1) * B]
                else:
                    r_ap = r_b[:, (blk - rf) * B : (blk - rf + 1) * B]
                if blk == 0 and first_segs:
                    sizes = list(first_segs)
                elif blk == bpc - 1 and last_segs:
                    sizes = list(last_segs)
                else:
                    sizes = [
                        min(tile_rows, rows - o) for o in range(0, rows, tile_rows)
                    ]
                assert sum(sizes) == rows
                segs, o = [], 0
                for sz in sizes:
                    segs.append((o, sz))
                    o += sz
                for si, (o, seg) in enumerate(segs):
                    wt_tile = wtp.tile([B, seg], w_dt, tag="wt")
                    nc.sync.dma_start(
                        wt_tile[:], wt[blk * B : (blk + 1) * B, o : o + seg]
                    )
                    orows = out_rows or seg
                    for oo in range(0, seg, orows):
                        ow = min(orows, seg - oo)
                        out_tile = outp.tile([B, ow], out_dt, tag="out")
                        # Ping-pong over PSUM group tiles: burst the group's
                        # matmuls back-to-back, then drain with copies
                        # alternating between scalar and vector.
                        for g0 in range(oo, oo + ow, gs):
                            gw = min(gs, oo + ow - g0)
                            ps = psp.tile([B, gs], F32)
                            for h0 in range(0, gw, hs):
                                hw_ = min(hs, gw - h0)
                                nc.tensor.matmul(
                                    ps[:, h0 : h0 + hw_],
                                    r_ap,
                                    wt_tile[:, g0 + h0 : g0 + h0 + hw_],
                                    start=True,
                                    stop=True,
                                )
                            for c0 in range(0, gw, cs):
                                cw = min(cs, gw - c0)
                                eng = copy_engines[ncopy % len(copy_engines)]
                                ncopy += 1
                                dst = out_tile[:, g0 - oo + c0 : g0 - oo + c0 + cw]
                                src = ps[:, c0 : c0 + cw]
                                if eng == "vector":
                                    nc.vector.tensor_copy(dst, src)
                                else:
                                    nc.scalar.copy(dst, src)
                        # out-stores ride a HWDGE ring (ACT by default)
                        st_eng = nc.sync if store_engine == "sync" else nc.scalar
                        st_eng.dma_start(
                            out_t[blk * B : (blk + 1) * B, o + oo : o + oo + ow],
                            out_tile[:],
                        )
    nc.compile()
    return nc


def kernel_impl(w, angles, trace=False, bass_kwargs=None, **spmd_kwargs):
    w = np.asarray(w)
    bass_kwargs = dict(bass_kwargs or {})
    w_dt = bass_kwargs.setdefault("w_dt", W_DT)
    r_dt = bass_kwargs.setdefault("r_dt", R_DT)
    out_dt = bass_kwargs.setdefault("out_dt", OUT_DT)
    delta = bass_kwargs.pop("delta", DELTA)

    Rm = _build_rotation_matrices(np.asarray(angles))  # [NB, B, B] f64
    if delta:
        Rm = Rm - np.eye(B)
    # r_host[c, blk*B + c'] = Rm[blk][c, c']  (contiguous per SBUF partition c)
    r_host = (
        np.ascontiguousarray(Rm.transpose(1, 0, 2)).reshape(B, NB * B)
        .astype(_NP[r_dt])
    )
    nc = _build_bass(**bass_kwargs)
    csz = BPC * B  # 1024 w-columns per core
    in_maps = [
        {
            "wt": np.ascontiguousarray(w[:, i * csz : (i + 1) * csz].T).astype(
                _NP[w_dt]
            ),
            "r": r_host[:, i * csz : (i + 1) * csz],
        }
        for i in range(NCORES)
    ]
    res = run_bass_kernel_spmd(
        nc, in_maps, core_ids=list(range(NCORES)), trace=trace, **spmd_kwargs
    )
    out = np.empty((O, IN_F), dtype=np.float32)
    for i in range(NCORES):
        sl = slice(i * csz, (i + 1) * csz)
        dev = np.asarray(res.results[i]["out_t"]).astype(np.float32).T
        out[:, sl] = (w[:, sl] + dev) if delta else dev
    return out, res


def kernel(w, angles):
    out, _ = kernel_impl(w, angles, trace=False)
    return out


# revision 27
# speedup vs baseline: 1.0519x; 1.0391x over previous
"""Trainium2 Bass kernel for BlockGivensRotation (w @ R, block-diagonal).

The reference applies, per 128-column block of w, 8 sequential sweeps of 127
adjacent-plane Givens rotations.  The composition of all 1016 rotations of a
block is a fixed 128x128 orthogonal matrix R_nb that depends only on `angles`,
so the whole op is `out[:, nb*128:(nb+1)*128] = w[:, nb*128:(nb+1)*128] @ R_nb`
- a block-diagonal matmul, ideal for the tensor engine.

Host side: compose R (tiny: 64x128x128, built in f64 from the 65K angles).
Device side: shard the 64 column-blocks across the 8 cores (8 blocks each).

The kernel is DMA-bound (the 8 cores together sit at the chip HBM roofline),
so bytes moved is the metric that matters.  Two tricks cut it 4x vs f32:

1. Delta form: R = I + (R-I), so out = w + w@(R-I).  The device computes only
   delta = w@(R-I); the host adds w back in f32.  Every quantization error on
   the device path is then scaled by ||R-I||_F/sqrt(128) ~= 0.39 instead of 1.
2. fp8 I/O: w streams in as float8_e3m4 (1 byte), delta streams out as
   float8_e3m4, with the stationary R-I in fp16 and f32 PSUM accumulation.
   Measured end-to-end rel err ~8e-3 against the f32 reference (gate: 2e-2).

Each core streams w.T tiles from DRAM (SP ring), matmuls with the per-block
stationary R-I, copies PSUM->SBUF with the copy work alternated between the
vector and scalar engines (the only two that can read PSUM; one alone would
be the bottleneck), and writes delta.T tiles back on the ACT ring.  w is fed
transposed so the contraction dim (block columns) lies on SBUF partitions
with fully contiguous DMA; the host transposes shards in/out.
"""

import numpy as np
import ml_dtypes

import concourse.bacc as bacc
import concourse.mybir as mybir
import concourse.tile as tile
from concourse.bass_utils import run_bass_kernel_spmd

O = 8192          # w rows
IN_F = 8192       # w cols
B = 128           # Givens block size
NB = IN_F // B    # 64 blocks
NCORES = 8
BPC = NB // NCORES  # 8 column-blocks per core
F32 = mybir.dt.float32

# Device dtypes (mybir, numpy) and mode.  delta=True runs the R-I form with
# the host adding w back; delta=False computes w@R directly on device.
W_DT = mybir.dt.float8e3
R_DT = mybir.dt.float16
OUT_DT = mybir.dt.float8e3
DELTA = True

_NP = {
    mybir.dt.float8e3: ml_dtypes.float8_e3m4,
    mybir.dt.float8e4: ml_dtypes.float8_e4m3,
    mybir.dt.float16: np.float16,
    mybir.dt.bfloat16: ml_dtypes.bfloat16,
    mybir.dt.float32: np.float32,
}


def _build_rotation_matrices(angles: np.ndarray) -> np.ndarray:
    """Compose the sweeps of adjacent Givens rotations into one 128x128
    matrix per block by applying the reference recurrence to the identity
    (in float64, rounded once at the end)."""
    nb, s, bm1 = angles.shape
    b = bm1 + 1
    ang = np.asarray(angles, dtype=np.float64)
    c = np.cos(ang)
    sn = np.sin(ang)
    R = np.broadcast_to(np.eye(b), (nb, b, b)).copy()  # [NB, basis row, col]
    for sweep in range(s):
        cs, ss = c[:, sweep, :], sn[:, sweep, :]
        carry = R[:, :, 0].copy()
        for i in range(bm1):
            col_j = R[:, :, i + 1]
            ci = cs[:, i][:, None]
            si = ss[:, i][:, None]
            R[:, :, i] = ci * carry - si * col_j
            carry = si * carry + ci * col_j
        R[:, :, b - 1] = carry
    return R


def _build_bass(
    rows=O,
    bpc=BPC,
    ncores=NCORES,
    tile_rows=8192,
    wt_bufs=6,
    out_bufs=4,
    r_first=2,
    first_segs=(512, 3584, 4096),
    last_segs=(4096, 3584, 512),
    copy_engines=("vector", "scalar"),
    w_dt=W_DT,
    r_dt=R_DT,
    out_dt=OUT_DT,
    group_banks=2,
    copy_banks=1,
    out_rows=None,
    store_engine="scalar",
    # Routing early w tiles over the ACT ring (act_first_segs=2) measured
    # ~6us WORSE: the extra descriptor-gen delays the scalar copy stream
    # more than the sequencer's earlier init-barrier exit helps.
    act_first_segs=0,
):
    """Per-core program over this core's `bpc` column-blocks of w:

        out_t[blk*B + c', r] = sum_c Rmi[blk][c, c'] * wt[blk*B + c, r]

    rows: w rows (full, 8192); tile_rows: rows per DMA tile;
    wt_bufs/out_bufs: pipeline depth; r_first: blocks of R in the first
    (small) R chunk so the first matmul isn't gated on the whole R slice;
    first_segs: tapered tiles for the first block so the PE starts on a
    tiny tile; last_segs: tapered tiles for the last block so the final
    store (which the kernel-end barrier waits on) is tiny;
    copy_engines: round-robin assignment for the PSUM->SBUF copies;
    group_banks: PSUM banks per ping-pong group tile.  The 8 PSUM banks
    are split into `8//group_banks` group tiles; each group runs
    `group_banks` back-to-back matmuls (hiding the ~173ns PE pipe-fill)
    gated on the WHOLE tile being drained, while the copy engines drain
    the other tile.  Whole-tile gating keeps the PE in burst mode - with
    per-bank gating the pipeline degenerates into a stable trickle where
    every matmul is released by a single copy and re-pays the pipe-fill.
    """
    nc = bacc.Bacc(
        "TRN2", target_bir_lowering=False, debug=False, num_devices=ncores
    )
    wt = nc.dram_tensor("wt", [bpc * B, rows], w_dt, kind="ExternalInput")
    r = nc.dram_tensor("r", [B, bpc * B], r_dt, kind="ExternalInput")
    out_t = nc.dram_tensor("out_t", [bpc * B, rows], out_dt, kind="ExternalOutput")

    hs = 512                    # moving free-dim per matmul (one PSUM bank f32)
    gs = hs * group_banks       # free-dim per PSUM group tile
    cs = hs * copy_banks        # free-dim per PSUM->SBUF copy

    with tile.TileContext(nc) as tc:
        with (
            tc.tile_pool(name="rp", bufs=1) as rp,
            tc.tile_pool(name="wtp", bufs=wt_bufs) as wtp,
            tc.tile_pool(name="outp", bufs=out_bufs) as outp,
            tc.tile_pool(name="psp", bufs=8 // group_banks, space="PSUM") as psp,
        ):
            # The ACT-ring sequencer exits the init barrier ~1.2us before
            # SP's, so the latency-critical first bytes (the first R chunk
            # and the first `act_first_segs` w tiles of block 0) are issued
            # there, in that order; the bulk w stream rides SP and enters
            # the shared 16 hardware DMA queues behind them.  The big
            # second R chunk is deferred until after those loads (it isn't
            # read until block `r_first`, ~20us in).
            rf = min(r_first, bpc)
            r_a = rp.tile([B, rf * B], r_dt, tag="ra")
            nc.scalar.dma_start(r_a[:], r[:, : rf * B])
            r_b = None
            if rf < bpc:
                r_b = rp.tile([B, (bpc - rf) * B], r_dt, tag="rb")
                if act_first_segs == 0:
                    nc.scalar.dma_start(r_b[:], r[:, rf * B :])
            ncopy = 0
            for blk in range(bpc):
                if blk < rf:
                    r_ap = r_a[:, blk * B : (blk + 1) * B]
                else:
                    r_ap = r_b[:, (blk - rf) * B : (blk - rf + 1) * B]
                if blk == 0 and first_segs:
                    sizes = list(first_segs)
                elif blk == bpc - 1 and last_segs:
                    sizes = list(last_segs)
                else:
                    sizes = [
                        min(tile_rows, rows - o) for o in range(0, rows, tile_rows)
                    ]
                assert sum(sizes) == rows
                segs, o = [], 0
                for sz in sizes:
                    segs.append((o, sz))
                    o += sz
                for si, (o, seg) in enumerate(segs):
                    wt_tile = wtp.tile([B, seg], w_dt, tag="wt")
                    early = blk == 0 and si < act_first_segs
                    (nc.scalar if early else nc.sync).dma_start(
                        wt_tile[:], wt[blk * B : (blk + 1) * B, o : o + seg]
                    )
                    if (
                        r_b is not None
                        and act_first_segs
                        and blk == 0
                        and si == min(act_first_segs, len(segs)) - 1
                    ):
                        nc.scalar.dma_start(r_b[:], r[:, rf * B :])
                    orows = out_rows or seg
                    for oo in range(0, seg, orows):
                        ow = min(orows, seg - oo)
                        out_tile = outp.tile([B, ow], out_dt, tag="out")
                        # Ping-pong over PSUM group tiles: burst the group's
                        # matmuls back-to-back, then drain with copies
                        # alternating between scalar and vector.
                        for g0 in range(oo, oo + ow, gs):
                            gw = min(gs, oo + ow - g0)
                            ps = psp.tile([B, gs], F32)
                            for h0 in range(0, gw, hs):
                                hw_ = min(hs, gw - h0)
                                nc.tensor.matmul(
                                    ps[:, h0 : h0 + hw_],
                                    r_ap,
                                    wt_tile[:, g0 + h0 : g0 + h0 + hw_],
                                    start=True,
                                    stop=True,
                                )
                            for c0 in range(0, gw, cs):
                                cw = min(cs, gw - c0)
                                eng = copy_engines[ncopy % len(copy_engines)]
                                ncopy += 1
                                dst = out_tile[:, g0 - oo + c0 : g0 - oo + c0 + cw]
                                src = ps[:, c0 : c0 + cw]
                                if eng == "vector":
                                    nc.vector.tensor_copy(dst, src)
                                else:
                                    nc.scalar.copy(dst, src)
                        # out-stores ride a HWDGE ring (ACT by default)
                        st_eng = nc.sync if store_engine == "sync" else nc.scalar
                        st_eng.dma_start(
                            out_t[blk * B : (blk + 1) * B, o + oo : o + oo + ow],
                            out_tile[:],
                        )
    nc.compile()
    return nc


def kernel_impl(w, angles, trace=False, bass_kwargs=None, **spmd_kwargs):
    w = np.asarray(w)
    bass_kwargs = dict(bass_kwargs or {})
    w_dt = bass_kwargs.setdefault("w_dt", W_DT)
    r_dt = bass_kwargs.setdefault("r_dt", R_DT)
    out_dt = bass_kwargs.setdefault("out_dt", OUT_DT)
    delta = bass_kwargs.pop("delta", DELTA)

    Rm = _build_rotation_matrices(np.asarray(angles))  # [NB, B, B] f64
    if delta:
        Rm = Rm - np.eye(B)
    # r_host[c, blk*B + c'] = Rm[blk][c, c']  (contiguous per SBUF partition c)
    r_host = (
        np.ascontiguousarray(Rm.transpose(1, 0, 2)).reshape(B, NB * B)
        .astype(_NP[r_dt])
    )
    nc = _build_bass(**bass_kwargs)
    csz = BPC * B  # 1024 w-columns per core
    in_maps = [
        {
            "wt": np.ascontiguousarray(w[:, i * csz : (i + 1) * csz].T).astype(
                _NP[w_dt]
            ),
            "r": r_host[:, i * csz : (i + 1) * csz],
        }
        for i in range(NCORES)
    ]
    res = run_bass_kernel_spmd(
        nc, in_maps, core_ids=list(range(NCORES)), trace=trace, **spmd_kwargs
    )
    out = np.empty((O, IN_F), dtype=np.float32)
    for i in range(NCORES):
        sl = slice(i * csz, (i + 1) * csz)
        dev = np.asarray(res.results[i]["out_t"]).astype(np.float32).T
        out[:, sl] = (w[:, sl] + dev) if delta else dev
    return out, res


def kernel(w, angles):
    out, _ = kernel_impl(w, angles, trace=False)
    return out
